# revision 14
# baseline (speedup 1.0000x reference)
"""Alias-free activation (StyleGAN3-style) Trainium2 Bass kernel.

Pipeline per image-channel: bias-add -> 2x zero-insertion upsample + 12x12 FIR
(pad 10, gain 4) -> leaky-relu(0.2)*sqrt(2) -> clamp +-256 -> 12x12 FIR +
2x downsample (pad 0).

Strategy: pure data parallel over the batch dim (8 images -> 8 NeuronCores).
Depthwise convs run on the TensorEngine as banded-Toeplitz matmuls in bf16.
Row taps live inside the banded weights; column taps are PSUM-accumulated
shifted-rhs matmuls. The small tail tiles (28 Y rows / 9 out rows) pack 3
column taps per weight via partition-triplicated staging (built by SBUF->SBUF
DMA on the idle GpSimd queue), cutting their full-width pass count 3x.
Y is stored as two column-parity planes so down-conv rhs reads are stride-1.
ScalarE applies leaky-relu (two Relu passes) while evacuating PSUM; DVE
clamps. Output stores are batched per down-tile; x loads go through SWDGE so
they never queue behind output stores.
"""
import numpy as np
import ml_dtypes

import concourse.bass as bass
import concourse.bacc as bacc
import concourse.tile as tile
from concourse import mybir
from concourse.bass_utils import run_bass_kernel_spmd

N_CORES = 8
C, H, W = 128, 128, 128
XPADW = 140         # 5 zero cols | 128 image | 7 zero cols
YJ = 132            # Y columns per column-parity plane (2*132 = 264 total)
QO = 127            # output spatial size
CB = 32             # channels per block
NBLK = C // CB
CG = 3              # up-conv channels per PSUM tile (3*132 = 396 <= 512 fp32)
DCG = 4             # down-conv channels per PSUM tile (4*127 = 508 <= 512)
SQ2 = float(np.sqrt(2.0))
ACT_SCALE = 4.0 * SQ2   # upsample gain (2*2) * leaky-relu sqrt(2) gain
CLAMP = 256.0

# (r0, rows): interleaved upsampled-row chunks, halo-overlapped so each down
# tile's 12-row stride-2 windows stay inside one chunk.
UP_CHUNKS = [(0, 128), (118, 128), (236, 28)]
# (t0, K, p0, M): down-conv input row tile [t0, t0+K) -> output rows [p0, p0+M)
DN_TILES = [(0, 128, 0, 59), (118, 128, 59, 59), (236, 28, 118, 9)]
DN_ORDER = [0, 2, 1]
# tail-tile tap packing: 3 dv-shifts per weight, pass bases per parity
PASS_B = {0: (-5, -2), 1: (-4, -1)}   # up: dv = B + g, g in 0..2
DN_BASES = (0, 3)                     # down: dd = base + g
T2_M0, T2_SPAN = 113, 15              # up tail: x rows 113..127
F32 = mybir.dt.float32
BF16 = mybir.dt.bfloat16


def _build_weights(fu: np.ndarray, fd: np.ndarray):
    """Banded Toeplitz weight banks.

    wup[:, (b*3+ci)*6+di, :rows]: [K=128 x-row, M=interleaved up-row k_local]
      Y[k, 2j+b] += sum_m x[m, j+dv] * fu[2m-k+10, 2*dv-b+10]
    wdn[:K, (ti*2+bp)*6+dd, :M]: [K=Y-row r_local, M=out-row p_local]
      Z[p, q] += sum_r Y[t0+r, 2(q+dd)+bp] * fd[t0+r-2p, 2dd+bp]
    wupt[g*15+ml, b*2+pi, kl]: up tail (chunk 2), 3 dv packed along K
    wdnt[g*28+rl, bp*2+pi, pl]: down tail (tile 2), 3 dd packed along K
    """
    wup = np.zeros((128, 24, 128), np.float32)
    for b in (0, 1):
        dvals = list(range(-5, 1)) if b == 0 else list(range(-4, 2))
        for ci, (r0, rows) in enumerate(UP_CHUNKS[:2]):
            for di, dv in enumerate(dvals):
                idx = (b * 2 + ci) * 6 + di
                m = np.arange(128)[:, None]
                kl = np.arange(rows)[None, :]
                frow = 2 * m - (r0 + kl) + 10
                valid = (frow >= 0) & (frow <= 11)
                wup[:, idx, :rows] = np.where(
                    valid, fu[np.clip(frow, 0, 11), 2 * dv - b + 10], 0.0)
    wupt = np.zeros((48, 4, 28), np.float32)
    for b in (0, 1):
        for pi in range(2):
            for g in range(3):
                dv = PASS_B[b][pi] + g
                m = (T2_M0 + np.arange(T2_SPAN))[:, None]
                kl = np.arange(28)[None, :]
                frow = 2 * m - (236 + kl) + 10
                valid = (frow >= 0) & (frow <= 11)
                wupt[g * T2_SPAN:(g + 1) * T2_SPAN, b * 2 + pi, :] = np.where(
                    valid, fu[np.clip(frow, 0, 11), 2 * dv - b + 10], 0.0)
    wdn = np.zeros((128, 24, 64), np.float32)
    for ti in (0, 1):
        t0, K, p0, M = DN_TILES[ti]
        for bp in (0, 1):
            for dd in range(6):
                idx = (ti * 2 + bp) * 6 + dd
                rl = np.arange(K)[:, None]
                pl = np.arange(M)[None, :]
                grow = t0 + rl - 2 * (p0 + pl)
                valid = (grow >= 0) & (grow <= 11)
                wdn[:K, idx, :M] = np.where(
                    valid, fd[np.clip(grow, 0, 11), 2 * dd + bp], 0.0)
    wdnt = np.zeros((96, 4, 9), np.float32)
    for bp in (0, 1):
        for pi in range(2):
            for g in range(3):
                dd = DN_BASES[pi] + g
                rl = np.arange(28)[:, None]
                pl = np.arange(9)[None, :]
                grow = 236 + rl - 2 * (118 + pl)
                valid = (grow >= 0) & (grow <= 11)
                wdnt[g * 28:(g + 1) * 28, bp * 2 + pi, :] = np.where(
                    valid, fd[np.clip(grow, 0, 11), 2 * dd + bp], 0.0)
    bf = ml_dtypes.bfloat16
    return wup.astype(bf), wdn.astype(bf), wupt.astype(bf), wdnt.astype(bf)


def _cgroups(step):
    out = []
    c0 = 0
    while c0 < CB:
        out.append((c0, min(step, CB - c0)))
        c0 += step
    return out


def _batched(groups, n):
    for i in range(0, len(groups), n):
        yield groups[i:i + n]


def _build_program() -> bacc.Bacc:
    nc = bacc.Bacc("TRN2", target_bir_lowering=False, debug=False,
                   num_devices=N_CORES)
    x_d = nc.dram_tensor("x", [C, H, W], F32, kind="ExternalInput")
    bias_d = nc.dram_tensor("bias", [C], F32, kind="ExternalInput")
    wup_d = nc.dram_tensor("wup", [128, 24, 128], BF16, kind="ExternalInput")
    wdn_d = nc.dram_tensor("wdn", [128, 24, 64], BF16, kind="ExternalInput")
    wupt_d = nc.dram_tensor("wupt", [48, 4, 28], BF16, kind="ExternalInput")
    wdnt_d = nc.dram_tensor("wdnt", [96, 4, 9], BF16, kind="ExternalInput")
    out_d = nc.dram_tensor("out", [C, QO, QO], F32, kind="ExternalOutput")

    groups = _cgroups(CG)
    dgroups = _cgroups(DCG)

    with tile.TileContext(nc) as tc:
        with (
            tc.tile_pool(name="consts", bufs=1) as consts,
            tc.tile_pool(name="xf32", bufs=2) as xf32p,
            tc.tile_pool(name="xin", bufs=2) as xin,
            tc.tile_pool(name="ybuf", bufs=1) as ybuf,
            tc.tile_pool(name="ustg", bufs=1) as ustgp,
            tc.tile_pool(name="dstg", bufs=1) as dstgp,
            tc.tile_pool(name="lstage", bufs=8) as lstage,
            tc.tile_pool(name="ostage", bufs=2) as ostage,
            tc.tile_pool(name="ps", bufs=4, space="PSUM") as psp,
        ):
            wup_sb = consts.tile([128, 24, 128], BF16)
            nc.sync.dma_start(out=wup_sb, in_=wup_d[:, :, :])
            wdn_sb = consts.tile([128, 24, 64], BF16)
            nc.sync.dma_start(out=wdn_sb, in_=wdn_d[:, :, :])
            wupt_sb = consts.tile([48, 4, 28], BF16)
            nc.sync.dma_start(out=wupt_sb, in_=wupt_d[:, :, :])
            wdnt_sb = consts.tile([96, 4, 9], BF16)
            nc.sync.dma_start(out=wdnt_sb, in_=wdnt_d[:, :, :])
            bias_sb = consts.tile([128, C], F32)
            b_ap = bias_d[:]
            bias_bcast = bass.AP(tensor=b_ap.tensor, offset=b_ap.offset,
                                 ap=[[0, 128]] + list(b_ap.ap))
            nc.sync.dma_start(out=bias_sb, in_=bias_bcast)

            def _prep_block(blk):
                # load x block as [h, c, w] via SWDGE (keeps the Sync queue
                # free for output stores); bias-add + cast to bf16
                ch0 = blk * CB
                xf = xf32p.tile([128, CB, W], F32)
                nc.gpsimd.dma_start(
                    out=xf,
                    in_=x_d[ch0:ch0 + CB, :, :].rearrange("c h w -> h c w"))
                x_t = xin.tile([128, CB, XPADW], BF16)
                nc.gpsimd.memset(x_t[:, :, 0:5], 0.0)
                nc.gpsimd.memset(x_t[:, :, 133:XPADW], 0.0)
                bsl = bias_sb[:, ch0:ch0 + CB]
                b_b = bass.AP(tensor=bsl.tensor, offset=bsl.offset,
                              ap=list(bsl.ap) + [[0, W]])
                nc.vector.tensor_add(x_t[:, :, 5:133], xf, b_b)
                return x_t

            def _evac(ps, y_t, rows, c0, ncg, b):
                # leaky-relu via two Relu passes: y = Relu(c*v) - Relu(-0.2c*v)
                ysl = y_t[:rows, c0:c0 + ncg, b, :]
                o2 = lstage.tile([128, CG, YJ], BF16, name="o2", tag="o2")
                nc.vector.tensor_scalar(
                    out=o2[:rows, :ncg, :], in0=ps[:rows, :ncg, :],
                    scalar1=0.0, scalar2=-0.2 * ACT_SCALE,
                    op0=mybir.AluOpType.min, op1=mybir.AluOpType.mult)
                nc.scalar.activation(
                    out=ysl, in_=ps[:rows, :ncg, :],
                    func=mybir.ActivationFunctionType.Relu,
                    scale=ACT_SCALE)
                nc.vector.tensor_tensor(ysl, ysl, o2[:rows, :ncg, :],
                                        mybir.AluOpType.subtract)

            x_next = _prep_block(0)
            for blk in range(NBLK):
                ch0 = blk * CB
                x_t = x_next

                # ---- upsample conv -> Y planes, lrelu+clamp ----
                ytiles = []
                for ci, (r0, rows) in enumerate(UP_CHUNKS):
                    y_t = ybuf.tile([128, CB, 2, YJ], BF16,
                                    name=f"y{ci}", tag=f"y{ci}")
                    ytiles.append(y_t)
                    if ci < 2:
                        for b in (0, 1):
                            for batch in _batched(groups, 4):
                                psl = [psp.tile([128, CG, YJ], F32,
                                                name="psu", tag="up")
                                       for _ in batch]
                                for di in range(6):
                                    wsl = wup_sb[:, (b * 2 + ci) * 6 + di,
                                                 :rows]
                                    dv = (di - 5) if b == 0 else (di - 4)
                                    for (c0, ncg), ps in zip(batch, psl):
                                        nc.tensor.matmul(
                                            ps[:rows, :ncg, :],
                                            wsl,
                                            x_t[:, c0:c0 + ncg,
                                                5 + dv:5 + dv + YJ],
                                            start=(di == 0), stop=(di == 5))
                                for (c0, ncg), ps in zip(batch, psl):
                                    _evac(ps, y_t, rows, c0, ncg, b)
                    else:
                        # tail chunk: 3 dv packed along K via staging
                        ust = ustgp.tile([48, CB, 138], BF16,
                                         name="ust", tag="ust")
                        for g in range(3):
                            nc.gpsimd.dma_start(
                                out=ust[g * T2_SPAN:(g + 1) * T2_SPAN, :, :],
                                in_=x_t[T2_M0:T2_M0 + T2_SPAN, :, g:g + 138])
                        for b in (0, 1):
                            for batch in _batched(groups, 4):
                                psl = [psp.tile([128, CG, YJ], F32,
                                                name="psu", tag="up")
                                       for _ in batch]
                                for pi in range(2):
                                    cb0 = 5 + PASS_B[b][pi]
                                    wsl = wupt_sb[:45, b * 2 + pi, :]
                                    for (c0, ncg), ps in zip(batch, psl):
                                        nc.tensor.matmul(
                                            ps[:rows, :ncg, :],
                                            wsl,
                                            ust[:45, c0:c0 + ncg,
                                                cb0:cb0 + YJ],
                                            start=(pi == 0), stop=(pi == 1))
                                for (c0, ncg), ps in zip(batch, psl):
                                    _evac(ps, y_t, rows, c0, ncg, b)
                    nc.vector.tensor_scalar(
                        out=y_t[:rows], in0=y_t[:rows],
                        scalar1=CLAMP, scalar2=-CLAMP,
                        op0=mybir.AluOpType.min, op1=mybir.AluOpType.max)

                # prefetch next block's input while the down conv runs
                if blk + 1 < NBLK:
                    x_next = _prep_block(blk + 1)

                # ---- downsample conv; batched store per tile ----
                for ti in DN_ORDER:
                    t0, K, p0, M = DN_TILES[ti]
                    y_t = ytiles[ti]
                    o_t = ostage.tile([128, CB, QO], F32,
                                      name="ot", tag="ot")
                    if ti == 2:
                        dst = dstgp.tile([96, CB, 2, 130], BF16,
                                         name="dst", tag="dst")
                        for g in range(3):
                            nc.gpsimd.dma_start(
                                out=dst[g * 28:(g + 1) * 28, :, :, :],
                                in_=y_t[0:28, :, :, g:g + 130])
                        for batch in _batched(dgroups, 4):
                            psl = [psp.tile([128, DCG, QO], F32,
                                            name="psd", tag="dn")
                                   for _ in batch]
                            idx = 0
                            for bp in (0, 1):
                                for pi in range(2):
                                    wsl = wdnt_sb[:84, bp * 2 + pi, :]
                                    jb = DN_BASES[pi]
                                    for (c0, ncg), ps in zip(batch, psl):
                                        nc.tensor.matmul(
                                            ps[:M, :ncg, :],
                                            wsl,
                                            dst[:84, c0:c0 + ncg, bp,
                                                jb:jb + QO],
                                            start=(idx == 0), stop=(idx == 3))
                                    idx += 1
                            for (c0, ncg), ps in zip(batch, psl):
                                nc.scalar.copy(
                                    o_t[:M, c0:c0 + ncg, :],
                                    ps[:M, :ncg, :])
                    else:
                        for batch in _batched(dgroups, 4):
                            psl = [psp.tile([128, DCG, QO], F32,
                                            name="psd", tag="dn")
                                   for _ in batch]
                            idx = 0
                            for bp in (0, 1):
                                for dd in range(6):
                                    wsl = wdn_sb[:K, (ti * 2 + bp) * 6 + dd,
                                                 :M]
                                    for (c0, ncg), ps in zip(batch, psl):
                                        nc.tensor.matmul(
                                            ps[:M, :ncg, :],
                                            wsl,
                                            y_t[:K, c0:c0 + ncg, bp,
                                                dd:dd + QO],
                                            start=(idx == 0),
                                            stop=(idx == 11))
                                    idx += 1
                            for (c0, ncg), ps in zip(batch, psl):
                                nc.scalar.copy(
                                    o_t[:M, c0:c0 + ncg, :],
                                    ps[:M, :ncg, :])
                    nc.sync.dma_start(
                        out=out_d[ch0:ch0 + CB, p0:p0 + M, :]
                        .rearrange("c p q -> p c q"),
                        in_=o_t[:M, :, :])
    nc.compile()
    return nc


_CACHE = {}


def kernel(input, bias, up_filter, down_filter):
    input = np.ascontiguousarray(np.asarray(input, dtype=np.float32))
    bias = np.ascontiguousarray(np.asarray(bias, dtype=np.float32))
    wup, wdn, wupt, wdnt = _build_weights(
        np.asarray(up_filter, np.float32), np.asarray(down_filter, np.float32))
    if "nc" not in _CACHE:
        _CACHE["nc"] = _build_program()
    nc = _CACHE["nc"]
    in_maps = [
        {"x": np.ascontiguousarray(input[i]), "bias": bias,
         "wup": wup, "wdn": wdn, "wupt": wupt, "wdnt": wdnt}
        for i in range(N_CORES)
    ]
    res = run_bass_kernel_spmd(nc, in_maps, core_ids=list(range(N_CORES)))
    globals()["_LAST_RESULT"] = res
    return np.stack([r["out"] for r in res.results], axis=0)


if __name__ == "__main__":
    rng = np.random.default_rng(0)
    out = kernel(rng.standard_normal((8, C, H, W), dtype=np.float32),
                 rng.standard_normal((C,), dtype=np.float32),
                 rng.random((12, 12), dtype=np.float32),
                 rng.random((12, 12), dtype=np.float32))
    print(out.shape, out.dtype)


# revision 17
# speedup vs baseline: 1.0978x; 1.0978x over previous
"""Alias-free activation (StyleGAN3-style) Trainium2 Bass kernel.

Pipeline per image-channel: bias-add -> 2x zero-insertion upsample + 12x12 FIR
(pad 10, gain 4) -> leaky-relu(0.2)*sqrt(2) -> clamp +-256 -> 12x12 FIR +
2x downsample (pad 0).

Strategy: pure data parallel over the batch dim (8 images -> 8 NeuronCores).
Depthwise convs run on the TensorEngine as banded-Toeplitz matmuls in bf16.
Row taps live inside the banded weights; column taps are PSUM-accumulated
shifted-rhs matmuls. The small tail tiles (28 Y rows / 9 out rows) pack 3
column taps per weight via partition-triplicated staging (built by SBUF->SBUF
DMA on the idle GpSimd queue), cutting their full-width pass count 3x.
Y is stored as two column-parity planes so down-conv rhs reads are stride-1.
ScalarE applies leaky-relu (two Relu passes) while evacuating PSUM; DVE
clamps. Output stores are batched per down-tile; x loads go through SWDGE so
they never queue behind output stores.
"""
import numpy as np
import ml_dtypes

import concourse.bass as bass
import concourse.bacc as bacc
import concourse.tile as tile
from concourse import mybir
from concourse.bass_utils import run_bass_kernel_spmd

N_CORES = 8
C, H, W = 128, 128, 128
XPADW = 140         # 5 zero cols | 128 image | 7 zero cols
YJ = 132            # Y columns per column-parity plane (2*132 = 264 total)
QO = 127            # output spatial size
CB = 32             # channels per block
NBLK = C // CB
CG = 3              # up-conv channels per PSUM tile (3*132 = 396 <= 512 fp32)
DCG = 4             # down-conv channels per PSUM tile (4*127 = 508 <= 512)
SQ2 = float(np.sqrt(2.0))
ACT_SCALE = 4.0 * SQ2   # upsample gain (2*2) * leaky-relu sqrt(2) gain
CLAMP = 256.0

# (r0, rows): interleaved upsampled-row chunks, halo-overlapped so each down
# tile's 12-row stride-2 windows stay inside one chunk.
UP_CHUNKS = [(0, 128), (118, 128), (236, 28)]
# (t0, K, p0, M): down-conv input row tile [t0, t0+K) -> output rows [p0, p0+M)
DN_TILES = [(0, 128, 0, 59), (118, 128, 59, 59), (236, 28, 118, 9)]
DN_ORDER = [0, 1, 2]
# tail-tile tap packing: 3 dv-shifts per weight, pass bases per parity
PASS_B = {0: (-5, -2), 1: (-4, -1)}   # up: dv = B + g, g in 0..2
DN_BASES = (0, 3)                     # down: dd = base + g
T2_M0, T2_SPAN = 113, 15              # up tail: x rows 113..127
F32 = mybir.dt.float32
BF16 = mybir.dt.bfloat16


def _build_weights(fu: np.ndarray, fd: np.ndarray):
    """Banded Toeplitz weight banks.

    wup[:, (b*3+ci)*6+di, :rows]: [K=128 x-row, M=interleaved up-row k_local]
      Y[k, 2j+b] += sum_m x[m, j+dv] * fu[2m-k+10, 2*dv-b+10]
    wdn[:K, (ti*2+bp)*6+dd, :M]: [K=Y-row r_local, M=out-row p_local]
      Z[p, q] += sum_r Y[t0+r, 2(q+dd)+bp] * fd[t0+r-2p, 2dd+bp]
    wupt[g*15+ml, b*2+pi, kl]: up tail (chunk 2), 3 dv packed along K
    wdnt[g*28+rl, bp*2+pi, pl]: down tail (tile 2), 3 dd packed along K
    """
    wup = np.zeros((128, 24, 128), np.float32)
    for b in (0, 1):
        dvals = list(range(-5, 1)) if b == 0 else list(range(-4, 2))
        for ci, (r0, rows) in enumerate(UP_CHUNKS[:2]):
            for di, dv in enumerate(dvals):
                idx = (b * 2 + ci) * 6 + di
                m = np.arange(128)[:, None]
                kl = np.arange(rows)[None, :]
                frow = 2 * m - (r0 + kl) + 10
                valid = (frow >= 0) & (frow <= 11)
                wup[:, idx, :rows] = np.where(
                    valid, fu[np.clip(frow, 0, 11), 2 * dv - b + 10], 0.0)
    wupt = np.zeros((48, 4, 28), np.float32)
    for b in (0, 1):
        for pi in range(2):
            for g in range(3):
                dv = PASS_B[b][pi] + g
                m = (T2_M0 + np.arange(T2_SPAN))[:, None]
                kl = np.arange(28)[None, :]
                frow = 2 * m - (236 + kl) + 10
                valid = (frow >= 0) & (frow <= 11)
                wupt[g * T2_SPAN:(g + 1) * T2_SPAN, b * 2 + pi, :] = np.where(
                    valid, fu[np.clip(frow, 0, 11), 2 * dv - b + 10], 0.0)
    wdn = np.zeros((128, 24, 64), np.float32)
    for ti in (0, 1):
        t0, K, p0, M = DN_TILES[ti]
        for bp in (0, 1):
            for dd in range(6):
                idx = (ti * 2 + bp) * 6 + dd
                rl = np.arange(K)[:, None]
                pl = np.arange(M)[None, :]
                grow = t0 + rl - 2 * (p0 + pl)
                valid = (grow >= 0) & (grow <= 11)
                wdn[:K, idx, :M] = np.where(
                    valid, fd[np.clip(grow, 0, 11), 2 * dd + bp], 0.0)
    wdnt = np.zeros((96, 4, 9), np.float32)
    for bp in (0, 1):
        for pi in range(2):
            for g in range(3):
                dd = DN_BASES[pi] + g
                rl = np.arange(28)[:, None]
                pl = np.arange(9)[None, :]
                grow = 236 + rl - 2 * (118 + pl)
                valid = (grow >= 0) & (grow <= 11)
                wdnt[g * 28:(g + 1) * 28, bp * 2 + pi, :] = np.where(
                    valid, fd[np.clip(grow, 0, 11), 2 * dd + bp], 0.0)
    bf = ml_dtypes.bfloat16
    return wup.astype(bf), wdn.astype(bf), wupt.astype(bf), wdnt.astype(bf)


def _cgroups(step):
    out = []
    c0 = 0
    while c0 < CB:
        out.append((c0, min(step, CB - c0)))
        c0 += step
    return out


def _batched(groups, n):
    for i in range(0, len(groups), n):
        yield groups[i:i + n]


def _build_program() -> bacc.Bacc:
    nc = bacc.Bacc("TRN2", target_bir_lowering=False, debug=False,
                   num_devices=N_CORES)
    x_d = nc.dram_tensor("x", [C, H, W], F32, kind="ExternalInput")
    bias_d = nc.dram_tensor("bias", [C], F32, kind="ExternalInput")
    wup_d = nc.dram_tensor("wup", [128, 24, 128], BF16, kind="ExternalInput")
    wdn_d = nc.dram_tensor("wdn", [128, 24, 64], BF16, kind="ExternalInput")
    wupt_d = nc.dram_tensor("wupt", [48, 4, 28], BF16, kind="ExternalInput")
    wdnt_d = nc.dram_tensor("wdnt", [96, 4, 9], BF16, kind="ExternalInput")
    out_d = nc.dram_tensor("out", [C, QO, QO], F32, kind="ExternalOutput")

    groups = _cgroups(CG)
    dgroups = _cgroups(DCG)

    with tile.TileContext(nc) as tc:
        with (
            tc.tile_pool(name="consts", bufs=1) as consts,
            tc.tile_pool(name="xf32", bufs=2) as xf32p,
            tc.tile_pool(name="xin", bufs=2) as xin,
            tc.tile_pool(name="ybuf", bufs=1) as ybuf,
            tc.tile_pool(name="ustg", bufs=1) as ustgp,
            tc.tile_pool(name="dstg", bufs=1) as dstgp,
            tc.tile_pool(name="lstage", bufs=8) as lstage,
            tc.tile_pool(name="ostage", bufs=3) as ostage,
            tc.tile_pool(name="ps", bufs=4, space="PSUM") as psp,
        ):
            wup_sb = consts.tile([128, 24, 128], BF16)
            nc.sync.dma_start(out=wup_sb, in_=wup_d[:, :, :])
            wdn_sb = consts.tile([128, 24, 64], BF16)
            nc.sync.dma_start(out=wdn_sb, in_=wdn_d[:, :, :])
            wupt_sb = consts.tile([48, 4, 28], BF16)
            nc.sync.dma_start(out=wupt_sb, in_=wupt_d[:, :, :])
            wdnt_sb = consts.tile([96, 4, 9], BF16)
            nc.sync.dma_start(out=wdnt_sb, in_=wdnt_d[:, :, :])
            bias_sb = consts.tile([128, C], F32)
            b_ap = bias_d[:]
            bias_bcast = bass.AP(tensor=b_ap.tensor, offset=b_ap.offset,
                                 ap=[[0, 128]] + list(b_ap.ap))
            nc.sync.dma_start(out=bias_sb, in_=bias_bcast)

            def _prep_block(blk):
                # load x block as [h, c, w] via SWDGE (keeps the Sync queue
                # free for output stores); bias-add + cast to bf16
                ch0 = blk * CB
                xf = xf32p.tile([128, CB, W], F32)
                nc.gpsimd.dma_start(
                    out=xf,
                    in_=x_d[ch0:ch0 + CB, :, :].rearrange("c h w -> h c w"))
                x_t = xin.tile([128, CB, XPADW], BF16)
                nc.gpsimd.memset(x_t[:, :, 0:5], 0.0)
                nc.gpsimd.memset(x_t[:, :, 133:XPADW], 0.0)
                bsl = bias_sb[:, ch0:ch0 + CB]
                b_b = bass.AP(tensor=bsl.tensor, offset=bsl.offset,
                              ap=list(bsl.ap) + [[0, W]])
                nc.vector.tensor_add(x_t[:, :, 5:133], xf, b_b)
                return x_t

            def _evac(ps, y_t, rows, c0, ncg, b):
                # leaky-relu via two Relu passes: y = Relu(c*v) - Relu(-0.2c*v)
                ysl = y_t[:rows, c0:c0 + ncg, b, :]
                o2 = lstage.tile([128, CG, YJ], BF16, name="o2", tag="o2")
                nc.scalar.activation(
                    out=o2[:rows, :ncg, :], in_=ps[:rows, :ncg, :],
                    func=mybir.ActivationFunctionType.Relu,
                    scale=-0.2 * ACT_SCALE)
                nc.scalar.activation(
                    out=ysl, in_=ps[:rows, :ncg, :],
                    func=mybir.ActivationFunctionType.Relu,
                    scale=ACT_SCALE)
                nc.vector.tensor_tensor(ysl, ysl, o2[:rows, :ncg, :],
                                        mybir.AluOpType.subtract)

            x_next = _prep_block(0)
            for blk in range(NBLK):
                ch0 = blk * CB
                x_t = x_next

                # ---- upsample conv -> Y planes, lrelu+clamp ----
                ytiles = []
                for ci, (r0, rows) in enumerate(UP_CHUNKS):
                    y_t = ybuf.tile([128, CB, 2, YJ], BF16,
                                    name=f"y{ci}", tag=f"y{ci}")
                    ytiles.append(y_t)
                    if ci < 2:
                        for b in (0, 1):
                            for batch in _batched(groups, 4):
                                psl = [psp.tile([128, CG, YJ], F32,
                                                name="psu", tag="up")
                                       for _ in batch]
                                for di in range(6):
                                    wsl = wup_sb[:, (b * 2 + ci) * 6 + di,
                                                 :rows]
                                    dv = (di - 5) if b == 0 else (di - 4)
                                    for (c0, ncg), ps in zip(batch, psl):
                                        nc.tensor.matmul(
                                            ps[:rows, :ncg, :],
                                            wsl,
                                            x_t[:, c0:c0 + ncg,
                                                5 + dv:5 + dv + YJ],
                                            start=(di == 0), stop=(di == 5))
                                for (c0, ncg), ps in zip(batch, psl):
                                    _evac(ps, y_t, rows, c0, ncg, b)
                    else:
                        # tail chunk: 3 dv packed along K via staging
                        ust = ustgp.tile([48, CB, 138], BF16,
                                         name="ust", tag="ust")
                        for g in range(3):
                            nc.gpsimd.dma_start(
                                out=ust[g * T2_SPAN:(g + 1) * T2_SPAN, :, :],
                                in_=x_t[T2_M0:T2_M0 + T2_SPAN, :, g:g + 138])
                        for b in (0, 1):
                            for batch in _batched(groups, 4):
                                psl = [psp.tile([128, CG, YJ], F32,
                                                name="psu", tag="up")
                                       for _ in batch]
                                for pi in range(2):
                                    cb0 = 5 + PASS_B[b][pi]
                                    wsl = wupt_sb[:45, b * 2 + pi, :]
                                    for (c0, ncg), ps in zip(batch, psl):
                                        nc.tensor.matmul(
                                            ps[:rows, :ncg, :],
                                            wsl,
                                            ust[:45, c0:c0 + ncg,
                                                cb0:cb0 + YJ],
                                            start=(pi == 0), stop=(pi == 1))
                                for (c0, ncg), ps in zip(batch, psl):
                                    _evac(ps, y_t, rows, c0, ncg, b)
                    nc.vector.tensor_scalar(
                        out=y_t[:rows], in0=y_t[:rows],
                        scalar1=CLAMP, scalar2=-CLAMP,
                        op0=mybir.AluOpType.min, op1=mybir.AluOpType.max)

                # prefetch next block's input while the down conv runs
                if blk + 1 < NBLK:
                    x_next = _prep_block(blk + 1)

                # ---- downsample conv; batched store per tile ----
                for ti in DN_ORDER:
                    t0, K, p0, M = DN_TILES[ti]
                    y_t = ytiles[ti]
                    o_t = ostage.tile([128, CB, QO], F32,
                                      name="ot", tag="ot")
                    if ti == 2:
                        dst = dstgp.tile([96, CB, 2, 130], BF16,
                                         name="dst", tag="dst")
                        for g in range(3):
                            nc.gpsimd.dma_start(
                                out=dst[g * 28:(g + 1) * 28, :, :, :],
                                in_=y_t[0:28, :, :, g:g + 130])
                        for batch in _batched(dgroups, 4):
                            psl = [psp.tile([128, DCG, QO], F32,
                                            name="psd", tag="dn")
                                   for _ in batch]
                            idx = 0
                            for bp in (0, 1):
                                for pi in range(2):
                                    wsl = wdnt_sb[:84, bp * 2 + pi, :]
                                    jb = DN_BASES[pi]
                                    for (c0, ncg), ps in zip(batch, psl):
                                        nc.tensor.matmul(
                                            ps[:M, :ncg, :],
                                            wsl,
                                            dst[:84, c0:c0 + ncg, bp,
                                                jb:jb + QO],
                                            start=(idx == 0), stop=(idx == 3))
                                    idx += 1
                            for (c0, ncg), ps in zip(batch, psl):
                                nc.scalar.copy(
                                    o_t[:M, c0:c0 + ncg, :],
                                    ps[:M, :ncg, :])
                    else:
                        for batch in _batched(dgroups, 4):
                            psl = [psp.tile([128, DCG, QO], F32,
                                            name="psd", tag="dn")
                                   for _ in batch]
                            idx = 0
                            for bp in (0, 1):
                                for dd in range(6):
                                    wsl = wdn_sb[:K, (ti * 2 + bp) * 6 + dd,
                                                 :M]
                                    for (c0, ncg), ps in zip(batch, psl):
                                        nc.tensor.matmul(
                                            ps[:M, :ncg, :],
                                            wsl,
                                            y_t[:K, c0:c0 + ncg, bp,
                                                dd:dd + QO],
                                            start=(idx == 0),
                                            stop=(idx == 11))
                                    idx += 1
                            for (c0, ncg), ps in zip(batch, psl):
                                nc.scalar.copy(
                                    o_t[:M, c0:c0 + ncg, :],
                                    ps[:M, :ncg, :])
                    nc.sync.dma_start(
                        out=out_d[ch0:ch0 + CB, p0:p0 + M, :]
                        .rearrange("c p q -> p c q"),
                        in_=o_t[:M, :, :])
    nc.compile()
    return nc


_CACHE = {}


def kernel(input, bias, up_filter, down_filter):
    input = np.ascontiguousarray(np.asarray(input, dtype=np.float32))
    bias = np.ascontiguousarray(np.asarray(bias, dtype=np.float32))
    wup, wdn, wupt, wdnt = _build_weights(
        np.asarray(up_filter, np.float32), np.asarray(down_filter, np.float32))
    if "nc" not in _CACHE:
        _CACHE["nc"] = _build_program()
    nc = _CACHE["nc"]
    in_maps = [
        {"x": np.ascontiguousarray(input[i]), "bias": bias,
         "wup": wup, "wdn": wdn, "wupt": wupt, "wdnt": wdnt}
        for i in range(N_CORES)
    ]
    res = run_bass_kernel_spmd(nc, in_maps, core_ids=list(range(N_CORES)))
    globals()["_LAST_RESULT"] = res
    return np.stack([r["out"] for r in res.results], axis=0)


if __name__ == "__main__":
    rng = np.random.default_rng(0)
    out = kernel(rng.standard_normal((8, C, H, W), dtype=np.float32),
                 rng.standard_normal((C,), dtype=np.float32),
                 rng.random((12, 12), dtype=np.float32),
                 rng.random((12, 12), dtype=np.float32))
    print(out.shape, out.dtype)
